# revision 37
# baseline (speedup 1.0000x reference)
"""Multi-head attention (B=4, S=2048, D=1024, H=16, Dh=64) on 8 TRN2 NeuronCores.

Sharding: core c handles batch b = c // 2 and head group g = c % 2 (8 heads
each).  Every core computes Q/K/V projections for its batch+heads, the
attention for those heads, and a *partial* output projection (its heads'
slice of Wo).  The host sums the two partials per batch while unsharding —
the tensor-parallel all-reduce on the output, done during gather.

v3 structure (all bf16 matmul operands, fp32 PSUM accumulation):
  - logits^T[t,f] per head pair via two K=64 matmuls issued back-to-back so
    they run CONCURRENTLY on disjoint PE row groups (rows 0-63 / 64-127).
  - one ScalarE Exp instruction per (m, head-pair): reads the [128, 2, 512]
    logits PSUM tile flat (FD=1024).  ScalarE does nothing else; its ~273us
    exp stream is the phase-D pacing constraint, so every other engine's
    work is scheduled to hide underneath it.
  - pl (logits PSUM) double-buffered so the m+1 logits pair never waits on
    the exp of m; e (exp output) 8-deep so the exp stream rides over the
    per-cc epilogue latency without stalling.
  - vt value tiles are padded to 128 columns with ONES in cols 64..127: the
    ctx matmul then yields ctx^T rows 0..63 plus 64 replicated denominator
    rows 64..127 in one [128, 512] output — full PE width, a 128-wide
    stationary operand (fast-weight-load eligible), free denom "broadcast".
  - softmax epilogue per (j, cc): lane-aligned [64,512] copy of the
    replicated denominator rows, partition-shift DMA to base-0, one
    [64,512] reciprocal_approx_fast, two [64,512] tensor_muls (head B via a
    base-0 temp + partition-shift DMA; DVE lanes are partition-hardwired).
  - everything except a minimal warmup (V tiles 0-3, q window 0, k windows
    0-1 for head pair 0) is emitted as deadline-tagged FILLER closures
    inside the attention m-loops: remaining V tiles and q/k windows for j0,
    then each next head pair's projections, and for j3 the output
    projection of already-finished f-windows.  Deadlines guarantee
    producers are emitted before in-order PE consumers; proportional pacing
    spreads the rest.  x^T inputs stream from DRAM in S/4 quarters
    (re-loaded per head pair; DMA bandwidth has big headroom).
"""

import sys

sys.path.insert(0, "/opt/trn_rl_repo")

import numpy as np
import ml_dtypes

BF = ml_dtypes.bfloat16

# Problem geometry (hardcoded; the harness always calls with these shapes).
B, S, D, H, Dh = 4, 2048, 1024, 16, 64
N_CORES = 8
H_LOC = H // 2          # heads per core
HK = H_LOC * Dh         # 512


class Cfg:
    def __init__(self, S=S, D=D, hloc=H_LOC, Dh=Dh):
        P = 128
        self.S, self.D, self.hloc, self.Dh = S, D, hloc, Dh
        self.P = P
        self.hk = hloc * Dh           # 512
        self.J = hloc // 2            # head pairs = 4
        self.DC = D // P              # contraction chunks = 8
        self.TT = S // P              # t (key) tiles = 16
        self.CW = 512                 # f-chunk width (one PSUM bank)
        self.NCC = S // self.CW       # f-chunks = 4
        self.scale = float(Dh) ** -0.5


def build_nc(cfg):
    import concourse.bass as bass
    import concourse.mybir as mybir
    import concourse.tile as tile
    from concourse import bacc
    from concourse.bass import ds, ts
    from contextlib import ExitStack

    FP32 = mybir.dt.float32
    BF16 = mybir.dt.bfloat16
    EXP = mybir.ActivationFunctionType.Exp

    P, Dh_, hloc = cfg.P, cfg.Dh, cfg.hloc
    S_, D_, hk = cfg.S, cfg.D, cfg.hk
    J, DC, TT, CW, NCC = cfg.J, cfg.DC, cfg.TT, cfg.CW, cfg.NCC
    NQ = S_ // CW                      # x quarters (= 4)

    nc = bacc.Bacc("TRN2")
    xq = nc.declare_dram_parameter("xq_t", [D_, S_], BF16, isOutput=False)
    xk = nc.declare_dram_parameter("xk_t", [D_, S_], BF16, isOutput=False)
    xv = nc.declare_dram_parameter("xv_t", [D_, S_], BF16, isOutput=False)
    wq = nc.declare_dram_parameter("wq", [D_, hk], BF16, isOutput=False)
    wk = nc.declare_dram_parameter("wk", [D_, hk], BF16, isOutput=False)
    wv = nc.declare_dram_parameter("wv", [D_, hk], BF16, isOutput=False)
    wo = nc.declare_dram_parameter("wo", [hk, D_], BF16, isOutput=False)
    out = nc.declare_dram_parameter("out_part", [S_, D_], FP32, isOutput=True)

    xs = {"q": xq, "k": xk, "v": xv}

    with tile.TileContext(nc) as tc, ExitStack() as ctx:
        singles = ctx.enter_context(tc.tile_pool(name="singles", bufs=1))

        # ---- persistent SBUF tensors -------------------------------------
        wq_sb = singles.tile([P, DC, hk], BF16, tag="wq", name="wq")
        wk_sb = singles.tile([P, DC, hk], BF16, tag="wk", name="wk")
        wv_sb = singles.tile([P, DC, hk], BF16, tag="wv", name="wv")
        wo_sb = singles.tile([P, J, D_], BF16, tag="wo", name="wo")
        qT = [singles.tile([P, S_], BF16, tag=f"qT{j}", name=f"qT{j}") for j in range(J)]
        kT = [singles.tile([P, S_], BF16, tag=f"kT{j}", name=f"kT{j}") for j in range(J)]
        ct = [singles.tile([P, S_], BF16, tag=f"ct{j}", name=f"ct{j}") for j in range(J)]
        # vt[m]: [t, head, 128]; cols 0..63 = V, cols 64..127 = 1.0 so the
        # ctx matmul replicates the softmax denominator on rows 64..127.
        vt = [singles.tile([P, hloc, P], BF16, tag=f"vt{m}", name=f"vt{m}")
              for m in range(TT)]

        w_sbs = {"q": wq_sb, "k": wk_sb, "v": wv_sb}

        with tc.tile_pool(name="xv_in", bufs=2) as xvpool, \
             tc.tile_pool(name="xqk_in", bufs=2) as xqkpool, \
             tc.tile_pool(name="psumP", bufs=2, space="PSUM") as pps, \
             tc.tile_pool(name="psumL", bufs=2, space="PSUM") as ppl, \
             tc.tile_pool(name="psumC", bufs=1, space="PSUM") as ppc, \
             tc.tile_pool(name="expp", bufs=8) as epool, \
             tc.tile_pool(name="rbc", bufs=2) as rpool, \
             tc.tile_pool(name="stage", bufs=2) as stpool, \
             tc.tile_pool(name="outb", bufs=3) as obpool:

            def load_w(which):
                w_dram = {"q": wq, "k": wk, "v": wv}[which]
                w_r = w_dram[:, :].rearrange("(a p) n -> p a n", p=P)
                for dc in range(DC):
                    nc.sync.dma_start(out=w_sbs[which][:, dc, :],
                                      in_=w_r[:, dc, :])

            def x_dma_fillers(which, quarter, eng=None, group=1):
                """Closures DMAing one S/4 quarter of an x^T input.  The
                sync sequencer pays ~0.6us per descriptor (shared with
                semaphore delivery), so deadline-relaxed prefetch loads use
                group=2 (half the descriptors, two dc chunks each; the
                transfer serializes 2x per queue, fine with >=40us slack).
                Warmup/tight loads keep group=1 for fastest landing."""
                pool = xvpool if which == "v" else xqkpool
                xt = pool.tile([P, DC, CW], BF16, tag=f"xt{which != 'v'}",
                               name=f"xt{which}{quarter}")
                src_r = xs[which][:, :].rearrange("(a p) s -> p a s", p=P)

                def mk(dc):
                    def emit():
                        (eng or nc.sync).dma_start(
                            out=xt[:, dc:dc + group, :],
                            in_=src_r[:, dc:dc + group,
                                      ds(quarter * CW, CW)])
                    return emit
                return xt, [mk(dc) for dc in range(0, DC, group)]

            def qk_proj_fillers(which, j, xt_q, w):
                """Closures projecting x quarter w -> qT/kT[j] window w."""
                dst = qT[j] if which == "q" else kT[j]
                w_sb = w_sbs[which]
                holder = []

                def mk_mm(dc):
                    def emit():
                        if dc == 0:
                            holder.append(pps.tile([P, CW], FP32,
                                                   tag="psP", name="psP"))
                        nc.tensor.matmul(
                            holder[0],
                            lhsT=w_sb[:, dc, ts(j, P)],
                            rhs=xt_q[:, dc, :],
                            start=(dc == 0), stop=(dc == DC - 1))
                    return emit

                def drain():
                    nc.vector.tensor_copy(out=dst[:, ds(w * CW, CW)],
                                          in_=holder[0])

                return [mk_mm(dc) for dc in range(DC)] + [drain]

            def v_proj_fillers(m, xt_q):
                """Closures producing the ones-padded vt[m] tile."""
                holder = []

                def mk_mm(dc):
                    def emit():
                        if dc == 0:
                            holder.append(pps.tile([P, hk], FP32,
                                                   tag="psP", name="psP"))
                        nc.tensor.matmul(holder[0],
                                         lhsT=xt_q[:, dc, ts(m % 4, P)],
                                         rhs=wv_sb[:, dc, :],
                                         start=(dc == 0), stop=(dc == DC - 1))
                    return emit

                def drain():
                    nc.vector.tensor_copy(
                        out=vt[m][:, :, 0:Dh_],
                        in_=holder[0].rearrange("p (h k) -> p h k", h=hloc))
                    nc.vector.memset(vt[m][:, :, Dh_:P], 1.0)

                return [mk_mm(dc) for dc in range(DC)] + [drain]

            def outproj_filler(ft, nd):
                """One output-projection group: [128,512] over all 4 j."""
                def emit():
                    po = pps.tile([P, CW], FP32, tag="psP", name="psP")
                    for j in range(J):
                        nc.tensor.matmul(
                            po,
                            lhsT=ct[j][:, ts(ft, P)],
                            rhs=wo_sb[:, j, ds(nd * CW, CW)],
                            start=(j == 0), stop=(j == J - 1))
                    ob = obpool.tile([P, CW], FP32, tag="ob", name="ob")
                    nc.vector.tensor_copy(out=ob, in_=po)
                    nc.sync.dma_start(out=out[ts(ft, P), ds(nd * CW, CW)],
                                      in_=ob)
                return emit

            # ---- attention ------------------------------------------------
            def lpair(j, cc, m, pl):
                # two K=64 logits matmuls on disjoint PE row groups
                for hh in range(2):
                    nc.tensor.matmul(
                        pl[:, hh, :],
                        lhsT=kT[j][hh * 64:(hh + 1) * 64, ts(m, P)],
                        rhs=qT[j][hh * 64:(hh + 1) * 64, ds(cc * CW, CW)],
                        start=True, stop=True)

            def attention_block(j, fillers, post_cc=None):
                """One head pair's attention, software-pipelined ACROSS cc
                boundaries: the next iteration's logits pair is always
                emitted immediately after this iteration's exp — before any
                filler or epilogue work — so the ScalarE exp stream never
                waits on lower-priority PE work.  fillers: list of
                (deadline_iter, closure) emitted once their deadline arrives
                or proportional pacing calls for them.  post_cc(cc) may
                append more fillers."""
                n_iter = NCC * TT
                state = {"done": 0, "appended": len(fillers)}

                def drain(it, pacing=True):
                    target = state["appended"] * (it + 1) // n_iter
                    while fillers and (fillers[0][0] <= it
                                       or (pacing and state["done"] < target)):
                        fillers.pop(0)[1]()
                        state["done"] += 1

                pcs = {}
                pl0 = ppl.tile([P, 2, CW], FP32, tag="pl", name="pl")
                lpair(j, 0, 0, pl0)
                pl1 = ppl.tile([P, 2, CW], FP32, tag="pl", name="pl")
                lpair(j, 0, 1, pl1)
                for gp in range(n_iter // 2):
                    sub = []
                    for g in (2 * gp, 2 * gp + 1):
                        cc, m = divmod(g, TT)
                        if m == 0:
                            pcs[cc] = (ppc.tile([P, CW], FP32, tag="pcA", name="pcA"),
                                       ppc.tile([P, CW], FP32, tag="pcB", name="pcB"))
                        e = epool.tile([P, 2, CW], BF16, tag="e", name="e")
                        nc.scalar.activation(out=e, in_=pl0 if g == 2 * gp else pl1,
                                             func=EXP, scale=cfg.scale)
                        sub.append((g, cc, m, e))
                    # both next logits pairs back-to-back: the second pair's
                    # weight loads hide under the first pair's row-disjoint
                    # matmuls, and the ctx weight-load stall is paid once per
                    # m-PAIR instead of once per m.
                    for g1 in (2 * gp + 2, 2 * gp + 3):
                        if g1 >= n_iter:
                            continue
                        cc1, m1 = divmod(g1, TT)
                        pl = ppl.tile([P, 2, CW], FP32, tag="pl", name="pl")
                        lpair(j, cc1, m1, pl)
                        if g1 == 2 * gp + 2:
                            pl0 = pl
                        else:
                            pl1 = pl
                    for g, cc, m, e in sub:
                        for hh in range(2):
                            nc.tensor.matmul(
                                pcs[cc][hh],
                                lhsT=vt[m][:, 2 * j + hh, :],
                                rhs=e[:, hh, :],
                                start=(m == 0), stop=(m == TT - 1))
                    g, cc, m, e = sub[-1]
                    if m == TT - 1:
                        # ---- softmax epilogue for (j, cc) ------------------
                        # Two plain copies release the pc PSUM banks at once
                        # (so the next cc's first ctx matmul never cascades
                        # into the exp stream); the rest runs lazily off the
                        # copies.  Partition shifts go through the gpsimd
                        # software DGE, keeping the Sync sequencer (which
                        # also delivers semaphores) free.
                        pcA, pcB = pcs.pop(cc)
                        # in the last block (the one running the output
                        # projection) no x prefetch competes for the sync
                        # DGE, and the shorter sync-DMA latency pulls the
                        # final f-window's epilogue off the critical tail.
                        dge = nc.sync if post_cc is not None else nc.gpsimd
                        stA = stpool.tile([P, CW], FP32, tag="stA", name="stA")
                        nc.vector.tensor_copy(out=stA, in_=pcA)
                        stB = stpool.tile([P, CW], FP32, tag="stB", name="stB")
                        nc.vector.tensor_copy(out=stB, in_=pcB)
                        dA = rpool.tile([64, CW], FP32, tag="dA", name="dA")
                        dge.dma_start(out=dA, in_=stA[64:128, :])
                        dB = rpool.tile([64, CW], FP32, tag="dB", name="dB")
                        dge.dma_start(out=dB, in_=stB[64:128, :])
                        rA = rpool.tile([64, CW], FP32, tag="rA", name="rA")
                        nc.vector.reciprocal_approx_fast(out=rA, in_=dA)
                        rB = rpool.tile([64, CW], FP32, tag="rB", name="rB")
                        nc.vector.reciprocal_approx_fast(out=rB, in_=dB)
                        nc.vector.tensor_mul(out=ct[j][0:64, ds(cc * CW, CW)],
                                             in0=stA[0:Dh_, :], in1=rA)
                        tmB = stpool.tile([64, CW], BF16, tag="tmB", name="tmB")
                        nc.vector.tensor_mul(out=tmB, in0=stB[0:Dh_, :], in1=rB)
                        dge.dma_start(out=ct[j][64:128, ds(cc * CW, CW)],
                                      in_=tmB)
                        if post_cc is not None:
                            for item in post_cc(cc):
                                fillers.append(item)
                                state["appended"] += 1
                    drain(g, pacing=(m not in (1, TT - 1)))
                while fillers:
                    fillers.pop(0)[1]()

            # ---- warmup: just enough for head pair 0 to start ------------
            # The sync sequencer writes one DMA descriptor per ~0.6us, so
            # the warmup is descriptor-count-bound: the V path (needed
            # first) gets interleaved per-chunk w/x descriptors; the q/k
            # loads use consolidated 2-chunk descriptors.  DMAs reusing a
            # ring slot are always emitted AFTER the previous occupant's
            # reads (both here and in filler-list order below).
            def x_dma2(which, quarter, eng=None):
                xt, dmas = x_dma_fillers(which, quarter, eng)
                for f in dmas:
                    f()
                return xt

            def load_w_eng(which, eng):
                w_dram = {"q": wq, "k": wk, "v": wv}[which]
                w_r = w_dram[:, :].rearrange("(a p) n -> p a n", p=P)
                for dc in range(DC):
                    eng.dma_start(out=w_sbs[which][:, dc, :],
                                  in_=w_r[:, dc, :])

            # Warmup descriptors split across the sync AND scalar HWDGEs
            # (ScalarE is idle until the first exp, ~40us in) so the
            # per-descriptor sequencer cost (~0.6us) halves in wall time.
            xt_vq = [None] * NQ
            xt_vq[0], vdmas = x_dma_fillers("v", 0)
            w_r_v = wv[:, :].rearrange("(a p) n -> p a n", p=P)
            for dc in range(DC):
                nc.sync.dma_start(out=wv_sb[:, dc, :], in_=w_r_v[:, dc, :])
                vdmas[dc]()
            xt_qq = [None] * NQ
            xt_qq[0] = x_dma2("q", 0, nc.scalar)
            load_w_eng("q", nc.scalar)
            xt_kq = [None] * NQ
            for m in range(2):
                for f in v_proj_fillers(m, xt_vq[0]):
                    f()
            xt_kq[0] = x_dma2("k", 0)
            load_w_eng("k", nc.scalar)
            for f in qk_proj_fillers("q", 0, xt_qq[0], 0):
                f()
            xt_kq[1] = x_dma2("k", 1)
            for f in qk_proj_fillers("k", 0, xt_kq[0], 0):
                f()
            nc.scalar.dma_start(
                out=wo_sb, in_=wo[:, :].rearrange("(j p) d -> p j d", p=P))

            # ---- j0 fillers: rest of V, rest of q/k(j0), with deadlines.
            # Constraints: V(m) by iter m-2; k window w by iter 4w-3; q
            # window w by iter 16w-3.  Deadlines non-decreasing in list
            # order (the drain pops strictly from the front).  Filler x
            # loads use the gpsimd DGE (see x_dma_fillers).
            # (measured: the gpsimd software DGE is far too slow for bulk x
            # loads — keep them on the sync HWDGE, spread by deadlines)
            gp_dge = None
            fl = []
            fl += [(0, f) for f in v_proj_fillers(2, xt_vq[0])]
            fl += [(0, f) for f in qk_proj_fillers("k", 0, xt_kq[1], 1)]
            fl += [(1, f) for f in v_proj_fillers(3, xt_vq[0])]
            xt_vq[1], dmas = x_dma_fillers("v", 1, gp_dge)
            fl += [(1, f) for f in dmas]
            fl += [(2, f) for f in v_proj_fillers(4, xt_vq[1])]
            fl += [(3, f) for f in v_proj_fillers(5, xt_vq[1])]
            xt_kq[2], dmas = x_dma_fillers("k", 2, gp_dge)
            fl += [(3, f) for f in dmas]
            fl += [(4, f) for f in qk_proj_fillers("k", 0, xt_kq[2], 2)]
            fl += [(4, f) for f in v_proj_fillers(6, xt_vq[1])]
            fl += [(5, f) for f in v_proj_fillers(7, xt_vq[1])]
            xt_vq[2], dmas = x_dma_fillers("v", 2, gp_dge)
            fl += [(5, f) for f in dmas]
            fl += [(6, f) for f in v_proj_fillers(8, xt_vq[2])]
            fl += [(7, f) for f in v_proj_fillers(9, xt_vq[2])]
            xt_kq[3], dmas = x_dma_fillers("k", 3, gp_dge)
            fl += [(7, f) for f in dmas]
            fl += [(8, f) for f in qk_proj_fillers("k", 0, xt_kq[3], 3)]
            fl += [(8, f) for f in v_proj_fillers(10, xt_vq[2])]
            xt_qq[2], dmas2 = x_dma_fillers("q", 2, gp_dge, group=2)
            fl += [(8, f) for f in dmas2]
            fl += [(9, f) for f in v_proj_fillers(11, xt_vq[2])]
            xt_vq[3], dmas = x_dma_fillers("v", 3, gp_dge)
            fl += [(9, f) for f in dmas]
            fl += [(10, f) for f in v_proj_fillers(12, xt_vq[3])]
            fl += [(11, f) for f in v_proj_fillers(13, xt_vq[3])]
            xt_qq[1], dmas = x_dma_fillers("q", 1, gp_dge)
            fl += [(11, f) for f in dmas]
            fl += [(12, f) for f in v_proj_fillers(14, xt_vq[3])]
            fl += [(12, f) for f in qk_proj_fillers("q", 0, xt_qq[1], 1)]
            fl += [(13, f) for f in v_proj_fillers(15, xt_vq[3])]
            fl += [(28, f) for f in qk_proj_fillers("q", 0, xt_qq[2], 2)]
            xt_qq[3], dmas3 = x_dma_fillers("q", 3, gp_dge)
            fl += [(29, f) for f in dmas3]
            fl += [(44, f) for f in qk_proj_fillers("q", 0, xt_qq[3], 3)]

            n_iter = NCC * TT
            for j in range(J):
                if j + 1 < J:
                    # next head pair's projections, due any time this block;
                    # spread deadlines keep the descriptor writes and DMA
                    # traffic smooth instead of bursty.
                    base = 0
                    for which in ("q", "k"):
                        for w in range(NQ):
                            xt_w, dmas = x_dma_fillers(which, w, gp_dge)
                            fl += [(min(base + i, n_iter - 1), f)
                                   for i, f in enumerate(dmas)]
                            fl += [(min(base + 4 + i, n_iter - 1), f)
                                   for i, f in enumerate(
                                       qk_proj_fillers(which, j + 1, xt_w, w))]
                            base += 7
                    post_cc = None
                else:
                    # j3: output projection of finished f-windows as fillers
                    def post_cc(cc):
                        return [(n_iter - 1, outproj_filler(ft, nd))
                                for ft in range(4 * cc, 4 * cc + 4)
                                for nd in range(D_ // CW)]
                attention_block(j, fl, post_cc)
                fl = []

    nc.compile()
    return nc


def shard_inputs(cfg, query_input, key_input, value_input, Wq, Wk, Wv, Wo):
    """Per-core input maps: core c -> batch c//2, head group c%2."""
    hloc = cfg.hloc
    in_maps = []
    for c in range(N_CORES):
        b, g = c // 2, c % 2
        hs = slice(g * hloc, (g + 1) * hloc)
        in_maps.append({
            "xq_t": np.ascontiguousarray(query_input[b].T).astype(BF),
            "xk_t": np.ascontiguousarray(key_input[b].T).astype(BF),
            "xv_t": np.ascontiguousarray(value_input[b].T).astype(BF),
            "wq": np.ascontiguousarray(Wq[:, hs, :]).reshape(cfg.D, cfg.hk).astype(BF),
            "wk": np.ascontiguousarray(Wk[:, hs, :]).reshape(cfg.D, cfg.hk).astype(BF),
            "wv": np.ascontiguousarray(Wv[:, hs, :]).reshape(cfg.D, cfg.hk).astype(BF),
            "wo": np.ascontiguousarray(Wo[hs]).reshape(cfg.hk, cfg.D).astype(BF),
        })
    return in_maps


_nc_cache = {}


def _get_nc(cfg):
    key = (cfg.S, cfg.D, cfg.hloc, cfg.Dh)
    if key not in _nc_cache:
        _nc_cache[key] = build_nc(cfg)
    return _nc_cache[key]


def run_spmd(inputs, trace=False, trace_cores=None):
    """Run the 8-core SPMD kernel; returns (output [B,S,D] fp32, BassKernelResults)."""
    from concourse.bass_utils import run_bass_kernel_spmd

    cfg = Cfg()
    nc = _get_nc(cfg)
    in_maps = shard_inputs(cfg, **{k: np.asarray(v) for k, v in inputs.items()})
    res = run_bass_kernel_spmd(nc, in_maps, list(range(N_CORES)),
                               trace=trace, trace_cores=trace_cores)
    out = np.empty((B, S, D), np.float32)
    for b in range(B):
        out[b] = res.results[2 * b]["out_part"] + res.results[2 * b + 1]["out_part"]
    return out, res


def kernel(**inputs):
    out, _ = run_spmd(inputs)
    return out


# revision 39
# speedup vs baseline: 1.0013x; 1.0013x over previous
"""Multi-head attention (B=4, S=2048, D=1024, H=16, Dh=64) on 8 TRN2 NeuronCores.

Sharding: core c handles batch b = c // 2 and head group g = c % 2 (8 heads
each).  Every core computes Q/K/V projections for its batch+heads, the
attention for those heads, and a *partial* output projection (its heads'
slice of Wo).  The host sums the two partials per batch while unsharding —
the tensor-parallel all-reduce on the output, done during gather.

v3 structure (all bf16 matmul operands, fp32 PSUM accumulation):
  - logits^T[t,f] per head pair via two K=64 matmuls issued back-to-back so
    they run CONCURRENTLY on disjoint PE row groups (rows 0-63 / 64-127).
  - one ScalarE Exp instruction per (m, head-pair): reads the [128, 2, 512]
    logits PSUM tile flat (FD=1024).  ScalarE does nothing else; its ~273us
    exp stream is the phase-D pacing constraint, so every other engine's
    work is scheduled to hide underneath it.
  - pl (logits PSUM) double-buffered so the m+1 logits pair never waits on
    the exp of m; e (exp output) 8-deep so the exp stream rides over the
    per-cc epilogue latency without stalling.
  - vt value tiles are padded to 128 columns with ONES in cols 64..127: the
    ctx matmul then yields ctx^T rows 0..63 plus 64 replicated denominator
    rows 64..127 in one [128, 512] output — full PE width, a 128-wide
    stationary operand (fast-weight-load eligible), free denom "broadcast".
  - softmax epilogue per (j, cc): lane-aligned [64,512] copy of the
    replicated denominator rows, partition-shift DMA to base-0, one
    [64,512] reciprocal_approx_fast, two [64,512] tensor_muls (head B via a
    base-0 temp + partition-shift DMA; DVE lanes are partition-hardwired).
  - everything except a minimal warmup (V tiles 0-3, q window 0, k windows
    0-1 for head pair 0) is emitted as deadline-tagged FILLER closures
    inside the attention m-loops: remaining V tiles and q/k windows for j0,
    then each next head pair's projections, and for j3 the output
    projection of already-finished f-windows.  Deadlines guarantee
    producers are emitted before in-order PE consumers; proportional pacing
    spreads the rest.  x^T inputs stream from DRAM in S/4 quarters
    (re-loaded per head pair; DMA bandwidth has big headroom).
"""

import sys

sys.path.insert(0, "/opt/trn_rl_repo")

import numpy as np
import ml_dtypes

BF = ml_dtypes.bfloat16

# Problem geometry (hardcoded; the harness always calls with these shapes).
B, S, D, H, Dh = 4, 2048, 1024, 16, 64
N_CORES = 8
H_LOC = H // 2          # heads per core
HK = H_LOC * Dh         # 512


class Cfg:
    def __init__(self, S=S, D=D, hloc=H_LOC, Dh=Dh):
        P = 128
        self.S, self.D, self.hloc, self.Dh = S, D, hloc, Dh
        self.P = P
        self.hk = hloc * Dh           # 512
        self.J = hloc // 2            # head pairs = 4
        self.DC = D // P              # contraction chunks = 8
        self.TT = S // P              # t (key) tiles = 16
        self.CW = 512                 # f-chunk width (one PSUM bank)
        self.NCC = S // self.CW       # f-chunks = 4
        self.scale = float(Dh) ** -0.5


def build_nc(cfg):
    import concourse.bass as bass
    import concourse.mybir as mybir
    import concourse.tile as tile
    from concourse import bacc
    from concourse.bass import ds, ts
    from contextlib import ExitStack

    FP32 = mybir.dt.float32
    BF16 = mybir.dt.bfloat16
    EXP = mybir.ActivationFunctionType.Exp

    P, Dh_, hloc = cfg.P, cfg.Dh, cfg.hloc
    S_, D_, hk = cfg.S, cfg.D, cfg.hk
    J, DC, TT, CW, NCC = cfg.J, cfg.DC, cfg.TT, cfg.CW, cfg.NCC
    NQ = S_ // CW                      # x quarters (= 4)

    nc = bacc.Bacc("TRN2")
    xq = nc.declare_dram_parameter("xq_t", [D_, S_], BF16, isOutput=False)
    xk = nc.declare_dram_parameter("xk_t", [D_, S_], BF16, isOutput=False)
    xv = nc.declare_dram_parameter("xv_t", [D_, S_], BF16, isOutput=False)
    wq = nc.declare_dram_parameter("wq", [D_, hk], BF16, isOutput=False)
    wk = nc.declare_dram_parameter("wk", [D_, hk], BF16, isOutput=False)
    wv = nc.declare_dram_parameter("wv", [D_, hk], BF16, isOutput=False)
    wo = nc.declare_dram_parameter("wo", [hk, D_], BF16, isOutput=False)
    out = nc.declare_dram_parameter("out_part", [S_, D_], FP32, isOutput=True)

    xs = {"q": xq, "k": xk, "v": xv}

    with tile.TileContext(nc) as tc, ExitStack() as ctx:
        singles = ctx.enter_context(tc.tile_pool(name="singles", bufs=1))

        # ---- persistent SBUF tensors -------------------------------------
        wq_sb = singles.tile([P, DC, hk], BF16, tag="wq", name="wq")
        wk_sb = singles.tile([P, DC, hk], BF16, tag="wk", name="wk")
        wv_sb = singles.tile([P, DC, hk], BF16, tag="wv", name="wv")
        wo_sb = singles.tile([P, J, D_], BF16, tag="wo", name="wo")
        qT = [singles.tile([P, S_], BF16, tag=f"qT{j}", name=f"qT{j}") for j in range(J)]
        kT = [singles.tile([P, S_], BF16, tag=f"kT{j}", name=f"kT{j}") for j in range(J)]
        ct = [singles.tile([P, S_], BF16, tag=f"ct{j}", name=f"ct{j}") for j in range(J)]
        # vt[m]: [t, head, 128]; cols 0..63 = V, cols 64..127 = 1.0 so the
        # ctx matmul replicates the softmax denominator on rows 64..127.
        vt = [singles.tile([P, hloc, P], BF16, tag=f"vt{m}", name=f"vt{m}")
              for m in range(TT)]

        w_sbs = {"q": wq_sb, "k": wk_sb, "v": wv_sb}

        with tc.tile_pool(name="xv_in", bufs=2) as xvpool, \
             tc.tile_pool(name="xqk_in", bufs=2) as xqkpool, \
             tc.tile_pool(name="psumP", bufs=2, space="PSUM") as pps, \
             tc.tile_pool(name="psumL", bufs=2, space="PSUM") as ppl, \
             tc.tile_pool(name="psumC", bufs=1, space="PSUM") as ppc, \
             tc.tile_pool(name="expp", bufs=8) as epool, \
             tc.tile_pool(name="rbc", bufs=2) as rpool, \
             tc.tile_pool(name="stage", bufs=2) as stpool, \
             tc.tile_pool(name="outb", bufs=3) as obpool:

            def load_w(which):
                w_dram = {"q": wq, "k": wk, "v": wv}[which]
                w_r = w_dram[:, :].rearrange("(a p) n -> p a n", p=P)
                for dc in range(DC):
                    nc.sync.dma_start(out=w_sbs[which][:, dc, :],
                                      in_=w_r[:, dc, :])

            def x_dma_fillers(which, quarter, eng=None, group=1):
                """Closures DMAing one S/4 quarter of an x^T input.  The
                sync sequencer pays ~0.6us per descriptor (shared with
                semaphore delivery), so deadline-relaxed prefetch loads use
                group=2 (half the descriptors, two dc chunks each; the
                transfer serializes 2x per queue, fine with >=40us slack).
                Warmup/tight loads keep group=1 for fastest landing."""
                pool = xvpool if which == "v" else xqkpool
                xt = pool.tile([P, DC, CW], BF16, tag=f"xt{which != 'v'}",
                               name=f"xt{which}{quarter}")
                src_r = xs[which][:, :].rearrange("(a p) s -> p a s", p=P)

                def mk(dc):
                    def emit():
                        (eng or nc.sync).dma_start(
                            out=xt[:, dc:dc + group, :],
                            in_=src_r[:, dc:dc + group,
                                      ds(quarter * CW, CW)])
                    return emit
                return xt, [mk(dc) for dc in range(0, DC, group)]

            def qk_proj_fillers(which, j, xt_q, w):
                """Closures projecting x quarter w -> qT/kT[j] window w."""
                dst = qT[j] if which == "q" else kT[j]
                w_sb = w_sbs[which]
                holder = []

                def mk_mm(dc):
                    def emit():
                        if dc == 0:
                            holder.append(pps.tile([P, CW], FP32,
                                                   tag="psP", name="psP"))
                        nc.tensor.matmul(
                            holder[0],
                            lhsT=w_sb[:, dc, ts(j, P)],
                            rhs=xt_q[:, dc, :],
                            start=(dc == 0), stop=(dc == DC - 1))
                    return emit

                def drain():
                    nc.vector.tensor_copy(out=dst[:, ds(w * CW, CW)],
                                          in_=holder[0])

                return [mk_mm(dc) for dc in range(DC)] + [drain]

            def v_proj_fillers(m, xt_q):
                """Closures producing the ones-padded vt[m] tile."""
                holder = []

                def mk_mm(dc):
                    def emit():
                        if dc == 0:
                            holder.append(pps.tile([P, hk], FP32,
                                                   tag="psP", name="psP"))
                        nc.tensor.matmul(holder[0],
                                         lhsT=xt_q[:, dc, ts(m % 4, P)],
                                         rhs=wv_sb[:, dc, :],
                                         start=(dc == 0), stop=(dc == DC - 1))
                    return emit

                def drain():
                    nc.vector.tensor_copy(
                        out=vt[m][:, :, 0:Dh_],
                        in_=holder[0].rearrange("p (h k) -> p h k", h=hloc))
                    nc.vector.memset(vt[m][:, :, Dh_:P], 1.0)

                return [mk_mm(dc) for dc in range(DC)] + [drain]

            def outproj_filler(ft, nd):
                """One output-projection group: [128,512] over all 4 j."""
                def emit():
                    po = pps.tile([P, CW], FP32, tag="psP", name="psP")
                    for j in range(J):
                        nc.tensor.matmul(
                            po,
                            lhsT=ct[j][:, ts(ft, P)],
                            rhs=wo_sb[:, j, ds(nd * CW, CW)],
                            start=(j == 0), stop=(j == J - 1))
                    ob = obpool.tile([P, CW], FP32, tag="ob", name="ob")
                    nc.vector.tensor_copy(out=ob, in_=po)
                    nc.sync.dma_start(out=out[ts(ft, P), ds(nd * CW, CW)],
                                      in_=ob)
                return emit

            # ---- attention ------------------------------------------------
            def lpair(j, cc, m, pl):
                # two K=64 logits matmuls on disjoint PE row groups
                for hh in range(2):
                    nc.tensor.matmul(
                        pl[:, hh, :],
                        lhsT=kT[j][hh * 64:(hh + 1) * 64, ts(m, P)],
                        rhs=qT[j][hh * 64:(hh + 1) * 64, ds(cc * CW, CW)],
                        start=True, stop=True)

            def attention_block(j, fillers, post_cc=None):
                """One head pair's attention, software-pipelined ACROSS cc
                boundaries: the next iteration's logits pair is always
                emitted immediately after this iteration's exp — before any
                filler or epilogue work — so the ScalarE exp stream never
                waits on lower-priority PE work.  fillers: list of
                (deadline_iter, closure) emitted once their deadline arrives
                or proportional pacing calls for them.  post_cc(cc) may
                append more fillers."""
                n_iter = NCC * TT
                state = {"done": 0, "appended": len(fillers)}

                def drain(it, pacing=True):
                    target = state["appended"] * (it + 1) // n_iter
                    while fillers and (fillers[0][0] <= it
                                       or (pacing and state["done"] < target)):
                        fillers.pop(0)[1]()
                        state["done"] += 1

                pcs = {}
                pl0 = ppl.tile([P, 2, CW], FP32, tag="pl", name="pl")
                lpair(j, 0, 0, pl0)
                pl1 = ppl.tile([P, 2, CW], FP32, tag="pl", name="pl")
                lpair(j, 0, 1, pl1)
                for gp in range(n_iter // 2):
                    sub = []
                    for g in (2 * gp, 2 * gp + 1):
                        cc, m = divmod(g, TT)
                        if m == 0:
                            pcs[cc] = (ppc.tile([P, CW], FP32, tag="pcA", name="pcA"),
                                       ppc.tile([P, CW], FP32, tag="pcB", name="pcB"))
                        e = epool.tile([P, 2, CW], BF16, tag="e", name="e")
                        nc.scalar.activation(out=e, in_=pl0 if g == 2 * gp else pl1,
                                             func=EXP, scale=cfg.scale)
                        sub.append((g, cc, m, e))
                    # both next logits pairs back-to-back: the second pair's
                    # weight loads hide under the first pair's row-disjoint
                    # matmuls, and the ctx weight-load stall is paid once per
                    # m-PAIR instead of once per m.
                    for g1 in (2 * gp + 2, 2 * gp + 3):
                        if g1 >= n_iter:
                            continue
                        cc1, m1 = divmod(g1, TT)
                        pl = ppl.tile([P, 2, CW], FP32, tag="pl", name="pl")
                        lpair(j, cc1, m1, pl)
                        if g1 == 2 * gp + 2:
                            pl0 = pl
                        else:
                            pl1 = pl
                    for g, cc, m, e in sub:
                        for hh in range(2):
                            nc.tensor.matmul(
                                pcs[cc][hh],
                                lhsT=vt[m][:, 2 * j + hh, :],
                                rhs=e[:, hh, :],
                                start=(m == 0), stop=(m == TT - 1))
                    g, cc, m, e = sub[-1]
                    if m == TT - 1:
                        # ---- softmax epilogue for (j, cc) ------------------
                        # Two plain copies release the pc PSUM banks at once
                        # (so the next cc's first ctx matmul never cascades
                        # into the exp stream); the rest runs lazily off the
                        # copies.  Partition shifts go through the gpsimd
                        # software DGE, keeping the Sync sequencer (which
                        # also delivers semaphores) free.
                        pcA, pcB = pcs.pop(cc)
                        # in the last block (the one running the output
                        # projection) no x prefetch competes for the sync
                        # DGE, and the shorter sync-DMA latency pulls the
                        # final f-window's epilogue off the critical tail.
                        dge = nc.sync if post_cc is not None else nc.gpsimd
                        stA = stpool.tile([P, CW], FP32, tag="stA", name="stA")
                        nc.vector.tensor_copy(out=stA, in_=pcA)
                        stB = stpool.tile([P, CW], FP32, tag="stB", name="stB")
                        nc.vector.tensor_copy(out=stB, in_=pcB)
                        dA = rpool.tile([64, CW], FP32, tag="dA", name="dA")
                        dge.dma_start(out=dA, in_=stA[64:128, :])
                        dB = rpool.tile([64, CW], FP32, tag="dB", name="dB")
                        dge.dma_start(out=dB, in_=stB[64:128, :])
                        rA = rpool.tile([64, CW], FP32, tag="rA", name="rA")
                        nc.vector.reciprocal_approx_fast(out=rA, in_=dA)
                        rB = rpool.tile([64, CW], FP32, tag="rB", name="rB")
                        nc.vector.reciprocal_approx_fast(out=rB, in_=dB)
                        nc.vector.tensor_mul(out=ct[j][0:64, ds(cc * CW, CW)],
                                             in0=stA[0:Dh_, :], in1=rA)
                        tmB = stpool.tile([64, CW], BF16, tag="tmB", name="tmB")
                        nc.vector.tensor_mul(out=tmB, in0=stB[0:Dh_, :], in1=rB)
                        dge.dma_start(out=ct[j][64:128, ds(cc * CW, CW)],
                                      in_=tmB)
                        if post_cc is not None:
                            for item in post_cc(cc):
                                fillers.append(item)
                                state["appended"] += 1
                    drain(g, pacing=(m not in (1, TT - 1)))
                while fillers:
                    fillers.pop(0)[1]()

            # ---- warmup: just enough for head pair 0 to start ------------
            # The sync sequencer writes one DMA descriptor per ~0.6us, so
            # the warmup is descriptor-count-bound: the V path (needed
            # first) gets interleaved per-chunk w/x descriptors; the q/k
            # loads use consolidated 2-chunk descriptors.  DMAs reusing a
            # ring slot are always emitted AFTER the previous occupant's
            # reads (both here and in filler-list order below).
            def x_dma2(which, quarter, eng=None):
                xt, dmas = x_dma_fillers(which, quarter, eng)
                for f in dmas:
                    f()
                return xt

            def load_w_eng(which, eng):
                w_dram = {"q": wq, "k": wk, "v": wv}[which]
                w_r = w_dram[:, :].rearrange("(a p) n -> p a n", p=P)
                for dc in range(DC):
                    eng.dma_start(out=w_sbs[which][:, dc, :],
                                  in_=w_r[:, dc, :])

            # Warmup descriptors split across the sync AND scalar HWDGEs
            # (ScalarE is idle until the first exp, ~40us in) so the
            # per-descriptor sequencer cost (~0.6us) halves in wall time.
            xt_vq = [None] * NQ
            xt_vq[0], vdmas = x_dma_fillers("v", 0)
            w_r_v = wv[:, :].rearrange("(a p) n -> p a n", p=P)
            for dc in range(DC):
                nc.sync.dma_start(out=wv_sb[:, dc, :], in_=w_r_v[:, dc, :])
                vdmas[dc]()
            xt_qq = [None] * NQ
            xt_qq[0] = x_dma2("q", 0, nc.scalar)
            load_w_eng("q", nc.scalar)
            xt_kq = [None] * NQ
            for m in range(2):
                for f in v_proj_fillers(m, xt_vq[0]):
                    f()
            xt_kq[0] = x_dma2("k", 0)
            load_w_eng("k", nc.scalar)
            for f in qk_proj_fillers("q", 0, xt_qq[0], 0):
                f()
            xt_kq[1] = x_dma2("k", 1)
            for f in qk_proj_fillers("k", 0, xt_kq[0], 0):
                f()
            nc.scalar.dma_start(
                out=wo_sb, in_=wo[:, :].rearrange("(j p) d -> p j d", p=P))
            # V(2,3) here fill the PE's DMA-wait gap between the warmup
            # projections and the first attention iteration (their xv/wv
            # inputs landed long ago), keeping HAM warm and thinning the
            # congested early-j0 filler drains.
            for m in (2, 3):
                for f in v_proj_fillers(m, xt_vq[0]):
                    f()

            # ---- j0 fillers: rest of V, rest of q/k(j0), with deadlines.
            # Constraints: V(m) by iter m-2; k window w by iter 4w-3; q
            # window w by iter 16w-3.  Deadlines non-decreasing in list
            # order (the drain pops strictly from the front).  Filler x
            # loads use the gpsimd DGE (see x_dma_fillers).
            # (measured: the gpsimd software DGE is far too slow for bulk x
            # loads — keep them on the sync HWDGE, spread by deadlines)
            gp_dge = None
            fl = []
            fl += [(0, f) for f in qk_proj_fillers("k", 0, xt_kq[1], 1)]
            xt_vq[1], dmas = x_dma_fillers("v", 1, gp_dge)
            fl += [(1, f) for f in dmas]
            fl += [(2, f) for f in v_proj_fillers(4, xt_vq[1])]
            fl += [(3, f) for f in v_proj_fillers(5, xt_vq[1])]
            xt_kq[2], dmas = x_dma_fillers("k", 2, gp_dge)
            fl += [(3, f) for f in dmas]
            fl += [(4, f) for f in qk_proj_fillers("k", 0, xt_kq[2], 2)]
            fl += [(4, f) for f in v_proj_fillers(6, xt_vq[1])]
            fl += [(5, f) for f in v_proj_fillers(7, xt_vq[1])]
            xt_vq[2], dmas = x_dma_fillers("v", 2, gp_dge)
            fl += [(5, f) for f in dmas]
            fl += [(6, f) for f in v_proj_fillers(8, xt_vq[2])]
            fl += [(7, f) for f in v_proj_fillers(9, xt_vq[2])]
            xt_kq[3], dmas = x_dma_fillers("k", 3, gp_dge)
            fl += [(7, f) for f in dmas]
            fl += [(8, f) for f in qk_proj_fillers("k", 0, xt_kq[3], 3)]
            fl += [(8, f) for f in v_proj_fillers(10, xt_vq[2])]
            xt_qq[2], dmas2 = x_dma_fillers("q", 2, gp_dge, group=2)
            fl += [(8, f) for f in dmas2]
            fl += [(9, f) for f in v_proj_fillers(11, xt_vq[2])]
            xt_vq[3], dmas = x_dma_fillers("v", 3, gp_dge)
            fl += [(9, f) for f in dmas]
            fl += [(10, f) for f in v_proj_fillers(12, xt_vq[3])]
            fl += [(11, f) for f in v_proj_fillers(13, xt_vq[3])]
            xt_qq[1], dmas = x_dma_fillers("q", 1, gp_dge)
            fl += [(11, f) for f in dmas]
            fl += [(12, f) for f in v_proj_fillers(14, xt_vq[3])]
            fl += [(12, f) for f in qk_proj_fillers("q", 0, xt_qq[1], 1)]
            fl += [(13, f) for f in v_proj_fillers(15, xt_vq[3])]
            fl += [(28, f) for f in qk_proj_fillers("q", 0, xt_qq[2], 2)]
            xt_qq[3], dmas3 = x_dma_fillers("q", 3, gp_dge)
            fl += [(29, f) for f in dmas3]
            fl += [(44, f) for f in qk_proj_fillers("q", 0, xt_qq[3], 3)]

            n_iter = NCC * TT
            for j in range(J):
                if j + 1 < J:
                    # next head pair's projections, due any time this block;
                    # spread deadlines keep the descriptor writes and DMA
                    # traffic smooth instead of bursty.
                    base = 0
                    for which in ("q", "k"):
                        for w in range(NQ):
                            xt_w, dmas = x_dma_fillers(which, w, gp_dge)
                            fl += [(min(base + i, n_iter - 1), f)
                                   for i, f in enumerate(dmas)]
                            fl += [(min(base + 4 + i, n_iter - 1), f)
                                   for i, f in enumerate(
                                       qk_proj_fillers(which, j + 1, xt_w, w))]
                            base += 7
                    post_cc = None
                else:
                    # j3: output projection of finished f-windows as fillers
                    def post_cc(cc):
                        return [(n_iter - 1, outproj_filler(ft, nd))
                                for ft in range(4 * cc, 4 * cc + 4)
                                for nd in range(D_ // CW)]
                attention_block(j, fl, post_cc)
                fl = []

    nc.compile()
    return nc


def shard_inputs(cfg, query_input, key_input, value_input, Wq, Wk, Wv, Wo):
    """Per-core input maps: core c -> batch c//2, head group c%2."""
    hloc = cfg.hloc
    in_maps = []
    for c in range(N_CORES):
        b, g = c // 2, c % 2
        hs = slice(g * hloc, (g + 1) * hloc)
        in_maps.append({
            "xq_t": np.ascontiguousarray(query_input[b].T).astype(BF),
            "xk_t": np.ascontiguousarray(key_input[b].T).astype(BF),
            "xv_t": np.ascontiguousarray(value_input[b].T).astype(BF),
            "wq": np.ascontiguousarray(Wq[:, hs, :]).reshape(cfg.D, cfg.hk).astype(BF),
            "wk": np.ascontiguousarray(Wk[:, hs, :]).reshape(cfg.D, cfg.hk).astype(BF),
            "wv": np.ascontiguousarray(Wv[:, hs, :]).reshape(cfg.D, cfg.hk).astype(BF),
            "wo": np.ascontiguousarray(Wo[hs]).reshape(cfg.hk, cfg.D).astype(BF),
        })
    return in_maps


_nc_cache = {}


def _get_nc(cfg):
    key = (cfg.S, cfg.D, cfg.hloc, cfg.Dh)
    if key not in _nc_cache:
        _nc_cache[key] = build_nc(cfg)
    return _nc_cache[key]


def run_spmd(inputs, trace=False, trace_cores=None):
    """Run the 8-core SPMD kernel; returns (output [B,S,D] fp32, BassKernelResults)."""
    from concourse.bass_utils import run_bass_kernel_spmd

    cfg = Cfg()
    nc = _get_nc(cfg)
    in_maps = shard_inputs(cfg, **{k: np.asarray(v) for k, v in inputs.items()})
    res = run_bass_kernel_spmd(nc, in_maps, list(range(N_CORES)),
                               trace=trace, trace_cores=trace_cores)
    out = np.empty((B, S, D), np.float32)
    for b in range(B):
        out[b] = res.results[2 * b]["out_part"] + res.results[2 * b + 1]["out_part"]
    return out, res


def kernel(**inputs):
    out, _ = run_spmd(inputs)
    return out
